# revision 20
# baseline (speedup 1.0000x reference)
"""Trainium2 Bass kernel for nn_MultiHeadAttention (B=2, T=2048, C=1024, H=16).

Sharding (8 cores): data-parallel over batch (2) x tensor-parallel over head
groups (4 groups of 4 heads), Megatron-style. Each core computes

    out_partial = softmax((x_b @ Wq_g.T) (x_b @ Wk_g.T).T / sqrt(d)) @ (x_b @ Wv_g.T) @ Wp_g.T

for its 4 heads; the 4 partials per batch are summed on the host (row-parallel
c_proj unshard) - no device collective needed.

Schedule notes:
  - g-OUTER phase order: the four g=0 (heads 0,1) attention phases run first,
    then the four g=1 phases. Only q01 (DMA-paced, with HAM keep-warm dummy
    matmuls filling the DMA gaps), k01 and v01 are projected up-front; the
    q23/k23/v23 projections are injected into the g=0 phases' tk slots, so
    the exp pipeline starts ~35us earlier and phase-B PE stalls are filled
    with useful projection work.
  - All main-PSUM pool slots are one bank ([128,512] fp32): the score tiles
    are allocated as per-head pairs so the row-tiled QK pair always lands in
    two different banks and the effective sc ring is 4 slots; cproj/unit/bc
    allocations only steal one bank at a time.
  - exp is emitted per head-half ([128,512]) with whole slots alternating
    between ACT and DVE (Schraudolph bf16 bit-trick on the DVE slots).
  - The softmax reciprocal runs on the otherwise-idle GpSimd engine as an
    exponent-flip bit-trick seed + two Newton-Raphson rounds on the packed
    dens2 tile (junk rows are kept at 1.0 so no NaN/Inf can leak through the
    broadcast matmul), freeing ~3.4us/phase of DVE time in the exp path.
  - outputs are written bf16 (host accumulates partials in fp32).
  - dummy matmuls on a junk SBUF tile keep the PE HAM clock-gate at 8/8
    through the DMA-paced head region and the serial tail.
"""

import numpy as np

import concourse.bass as bass
import concourse.mybir as mybir
import concourse.tile as tile
from concourse import bass_utils

F32 = mybir.dt.float32
F32R = mybir.dt.float32r
BF16 = mybir.dt.bfloat16
I16 = mybir.dt.int16
I32 = mybir.dt.int32


def legalize_waits(nc, max_waits=1):
    """Walrus codegen in this toolchain rejects instructions carrying more
    than one sync wait. Split extra waits into preceding same-engine NoOps
    at the BIR-JSON level and pin the serialized module on the nc object."""
    import json as _json
    d = _json.loads(nc.to_json_bytes())
    ctr = 0
    for fn in d.get("functions", []):
        for blk in fn.get("blocks", []) or []:
            insts = blk.get("instructions")
            if not insts:
                continue
            out = []
            for inst in insts:
                si = inst.get("sync_info")
                waits = (si or {}).get("on_wait") or []
                if len(waits) > max_waits:
                    keep, extra = waits[:max_waits], waits[max_waits:]
                    for w in extra:
                        ctr += 1
                        out.append({
                            "debug": inst.get("debug", 0),
                            "engine": inst["engine"],
                            "ins": [],
                            "outs": [],
                            "name": f"I-wsplit-{ctr}",
                            "opcode": "NoOp",
                            "sync_info": {"on_wait": [w], "on_update": []},
                        })
                    si["on_wait"] = keep
                out.append(inst)
            blk["instructions"] = out
    raw = _json.dumps(d).encode()
    nc.to_json_bytes = lambda: raw
    return nc

# Problem constants
B, T_FULL, C_FULL = 2, 2048, 1024
H_GLOBAL = 16
D = 64  # head dim
N_CORES = 8
HL = 4  # heads per core
CLOC = HL * D  # 256 local channels


def emit_mha_kernel(tc, out, xT, wqk, wv, wp, sel, zed, ident, T, C):
    """Emit the per-core MHA kernel into TileContext tc.

    out: dram [T, C] bf16 (partial output)
    xT:  dram [C, T]
    wqk: dram [128, 4*CT*128]  (ft-major: q01, q23, k01, k23)
    wv:  dram [128, 2*CT*128]  (ft-major: v01, v23)
    wp:  dram [128, KK*C]
    sel: dram [128, 128] (bc selector); zed: dram [128,512] of ONES
    """
    nc = tc.nc
    sel = sel.bitcast(F32R)
    zed = zed.bitcast(F32R)

    CT = C // 128          # c (contraction) tiles for projections
    TT = T // 128          # t tiles
    KK = CLOC // 128       # c_loc tiles (2)
    SUB = 512              # projection unit free width (one PSUM bank fp32)
    NU = T // SUB          # projection units per f-tile (4)
    DE = D + 1             # v columns incl. ones
    QB = min(512, T)       # tq block per head in phase B
    NQB = T // QB
    NPH = NQB * (HL // 2)  # attention phases: 8
    FTW = CT * 128         # columns per ft slice of wqk/wv (1024)

    import contextlib
    stack = contextlib.ExitStack()

    persist = stack.enter_context(tc.tile_pool(name="persist", bufs=1))
    main_ps = stack.enter_context(tc.tile_pool(name="main_ps", bufs=4, space="PSUM"))
    y_ps = stack.enter_context(tc.tile_pool(name="y_ps", bufs=4, space="PSUM"))
    exp_pool = stack.enter_context(tc.tile_pool(name="exp_pool", bufs=4))
    out_pool = stack.enter_context(tc.tile_pool(name="out_pool", bufs=2))
    small_pool = stack.enter_context(tc.tile_pool(name="small_pool", bufs=3))

    # ---- persistent SBUF tensors ----
    xt_sb = persist.tile([128, CT * T], BF16, name="xt_sb")
    wqk_sb = persist.tile([128, 4 * FTW], BF16, name="wqk_sb")
    wv_sb = persist.tile([128, 2 * FTW], BF16, name="wv_sb")
    wp_sb = persist.tile([128, KK * C], BF16, name="wp_sb")
    qk_sb = persist.tile([128, 4 * T], BF16, name="qk_sb")
    v_sb = persist.tile([128, TT * HL * DE], BF16, name="v_sb")
    vt_sb = persist.tile([128, 2 * T], BF16, name="vt_sb")
    yT_sb = persist.tile([128, KK * T], BF16, name="yT_sb")
    sel_sb = persist.tile([128, 128], F32R, name="sel_sb")
    dens2 = persist.tile([128, QB], F32R, name="dens2")
    ident_sb = persist.tile([128, 128], BF16, name="ident_sb")
    junk_sb = persist.tile([128, 512], BF16, name="junk_sb")

    def mm(out_ap, lhsT, rhs, **kw):
        nc.tensor.matmul(out_ap, lhsT, rhs, **kw)

    # ---- warm up the ACT exp table during the input DMA window ----
    warm = small_pool.tile([128, 1], F32, name="warm")
    const0 = nc.const_aps.aps[(mybir.dt.float32, 0.0)]
    nc.scalar.activation(warm[:], const0, mybir.ActivationFunctionType.Exp)

    # junk tile for HAM keep-warm dummy matmuls (no DMA dependency)
    nc.vector.memset(junk_sb[:], 0.0)

    dummy_state = {"tile": None}

    def new_dummy_tile():
        dummy_state["tile"] = main_ps.tile([128, 512], F32, name="dmy", tag="m")

    def dummy_mm(n=1):
        dp = dummy_state["tile"]
        for _ in range(n):
            mm(dp[:], junk_sb[:, 0:128], junk_sb[:, 0:512], start=True, stop=True)

    # ---- input DMAs: small tables first, then the critical prefix
    # (wqk ft0 -> xT -> wqk ft2 -> wv ft0), then the deferred weights. ----
    nc.sync.dma_start(ident_sb[:], ident[:])
    nc.sync.dma_start(sel_sb[:], sel[:])
    nc.sync.dma_start(dens2[:], zed[:])
    nc.vector.memset(
        v_sb[:].rearrange("p (t h e) -> p t h e", t=TT, h=HL)[:, :, :, D:DE], 1.0
    )
    nc.sync.dma_start(wqk_sb[:, 0:FTW], wqk[:, 0:FTW])  # q01
    nc.sync.dma_start(wqk_sb[:, 2 * FTW:3 * FTW], wqk[:, 2 * FTW:3 * FTW])  # k01
    for ct in range(CT):
        nc.sync.dma_start(
            xt_sb[:, ct * T:(ct + 1) * T], xT[ct * 128:(ct + 1) * 128, :]
        )
    nc.sync.dma_start(wv_sb[:, 0:FTW], wv[:, 0:FTW])  # v01
    nc.sync.dma_start(wqk_sb[:, FTW:2 * FTW], wqk[:, FTW:2 * FTW])  # q23
    nc.sync.dma_start(wqk_sb[:, 3 * FTW:4 * FTW], wqk[:, 3 * FTW:4 * FTW])  # k23
    nc.sync.dma_start(wv_sb[:, FTW:2 * FTW], wv[:, FTW:2 * FTW])  # v23
    nc.sync.dma_start(wp_sb[:], wp[:])

    # ---- A-head ----
    def w_lhsT(kind, ft, ct):
        if kind == "qk":
            return wqk_sb[:, ft * FTW + ct * 128: ft * FTW + (ct + 1) * 128]
        return wv_sb[:, ft * FTW + ct * 128: ft * FTW + (ct + 1) * 128]

    def proj_dst(kind, ft):
        if kind == "qk":
            return qk_sb, ft * T
        return vt_sb, ft * T

    new_dummy_tile()
    dummy_mm(16)  # ~3.4us of PE busy to flip HAM to 8/8 during the DMA lead-in

    # q01 + k01 together, ct-outer: 8 accumulation groups fill all 8 PSUM
    # banks and the matmuls stream densely behind the xT DMA (no filler
    # needed once the first chunks land).
    g_q = [y_ps.tile([128, 512], F32, name="ahead", tag="y") for _ in range(NU)]
    g_k = [main_ps.tile([128, 512], F32, name="aheadk", tag="m") for _ in range(NU)]
    for ct in range(CT):
        for u in range(NU):
            mm(
                g_q[u][:],
                w_lhsT("qk", 0, ct),
                xt_sb[:, ct * T + u * SUB: ct * T + (u + 1) * SUB],
                start=(ct == 0),
                stop=(ct == CT - 1),
            )
        for u in range(NU):
            mm(
                g_k[u][:],
                w_lhsT("qk", 2, ct),
                xt_sb[:, ct * T + u * SUB: ct * T + (u + 1) * SUB],
                start=(ct == 0),
                stop=(ct == CT - 1),
            )
    for u in range(NU):
        nc.vector.tensor_copy(qk_sb[:, u * SUB: (u + 1) * SUB], g_q[u][:])
        nc.scalar.copy(qk_sb[:, 2 * T + u * SUB: 2 * T + (u + 1) * SUB], g_k[u][:])

    def emit_proj_ftile(kind, ft):
        """One full f-tile projection (4 units x 8 ct), PE-bound, via y_ps."""
        groups = [y_ps.tile([128, 512], F32, name="ahead", tag="y") for _ in range(NU)]
        for ct in range(CT):
            for u in range(NU):
                mm(
                    groups[u][:],
                    w_lhsT(kind, ft, ct),
                    xt_sb[:, ct * T + u * SUB: ct * T + (u + 1) * SUB],
                    start=(ct == 0),
                    stop=(ct == CT - 1),
                )
        dst, col = proj_dst(kind, ft)
        for u in range(NU):
            nc.vector.tensor_copy(
                dst[:, col + u * SUB: col + (u + 1) * SUB], groups[u][:]
            )

    emit_proj_ftile("v", 0)   # v01

    def emit_unit(kind, ft, u, copy_dve):
        """One projection unit (8 ct matmuls into one PSUM bank + copy),
        injected into a phase-B tk slot."""
        ps = main_ps.tile([128, 512], F32, name="unit", tag="m")
        for ct in range(CT):
            mm(
                ps[:],
                w_lhsT(kind, ft, ct),
                xt_sb[:, ct * T + u * SUB: ct * T + (u + 1) * SUB],
                start=(ct == 0),
                stop=(ct == CT - 1),
            )
        dst, col = proj_dst(kind, ft)
        dap = dst[:, col + u * SUB: col + (u + 1) * SUB]
        if copy_dve:
            nc.vector.tensor_copy(dap, ps[:])
        else:
            nc.scalar.copy(dap, ps[:])

    # A-leftover unit schedule: (phase, slot) -> list of (kind, ft, u)
    unit_sched = {}
    for i in range(NU):
        unit_sched.setdefault((1, 3 if i < 2 else 11), []).append(("qk", 1, i))
        unit_sched.setdefault((2, 3 if i < 2 else 11), []).append(("qk", 3, i))
        unit_sched.setdefault((3, 3 if i < 2 else 11), []).append(("v", 1, i))

    def emit_vtrans(kf, tt):
        """PE-transpose one 128x128 tile of vt_sb (half kf) into v_sb."""
        tp = main_ps.tile([128, 128], BF16, name="tp", tag="m")
        nc.tensor.transpose(
            tp[:, 0:128],
            vt_sb[:, kf * T + tt * 128: kf * T + (tt + 1) * 128],
            ident_sb[:],
        )
        for hh in range(2):
            h = kf * 2 + hh
            nc.vector.tensor_copy(
                v_sb[:, tt * HL * DE + h * DE: tt * HL * DE + h * DE + D],
                tp[:, hh * 64: hh * 64 + 64],
            )

    # vtrans schedule: (phase, slot) -> list of (kf, tt). Tiles are emitted
    # (after that slot's AV) at least two slots before their first AV read.
    vt_sched = {}
    emit_vtrans(0, 0)
    emit_vtrans(0, 1)
    for s in range(0, 14, 2):   # phase 0, kf=0: tiles s+2, s+3 at slot s
        vt_sched[(0, s)] = [(0, s + 2), (0, s + 3)]
    vt_sched[(3, 12)] = [(1, 0), (1, 1)]
    vt_sched[(3, 14)] = [(1, 2), (1, 3)]
    for s in range(0, 12, 2):   # phase NQB, kf=1: tiles s+4, s+5 at slot s
        vt_sched[(NQB, s)] = [(1, s + 4), (1, s + 5)]

    # ---- Phase B: attention. Phase p: g = p // NQB, tb = p % NQB. ----
    ypairs = [None] * NPH
    scale = 1.0 / np.sqrt(D)

    def phase_gtb(p):
        return p // NQB, p % NQB

    def emit_qk(p, tk):
        g, tb = phase_gtb(p)
        qcol = g * T
        kcol = (2 + g) * T
        sc = []
        for i in range(2):
            p0 = i * 64
            sch = main_ps.tile([128, QB], F32, name="sc", tag="m")
            mm(
                sch[:],
                qk_sb[p0:p0 + 64, kcol + tk * 128: kcol + (tk + 1) * 128],
                qk_sb[p0:p0 + 64, qcol + tb * QB: qcol + (tb + 1) * QB],
                start=True,
                stop=True,
            )
            sc.append(sch)
        return sc

    # Schraudolph bf16 exp on the DVE for some slots per phase: bf16 bits of
    # exp(scale*s) ~= round(s*(scale*128/ln2) + (127*128 - 7.42)). The uniform
    # half-bit rounding bias cancels between numerator and denominator of the
    # softmax; residual noise is ~1.5% rms on the offloaded tiles.
    SCH_A = float(scale * 128.0 / np.log(2.0))
    SCH_B = float(127.0 * 128.0 - 7.42 + 0.5)
    DVE_SLOTS = (1, 4, 6, 9, 12, 15)

    def emit_exp(tk, sc):
        """exp of slot tk's scores, emitted one slot EARLY so the AV pair
        never head-of-line blocks the PE queue (keeps LDWEIGHTS hidden)."""
        halves = []
        if tk in DVE_SLOTS:
            et_i = exp_pool.tile([128, 2 * QB], I16, name="et", tag="et")
            for i in range(2):
                nc.vector.tensor_scalar(
                    et_i[:, i * QB:(i + 1) * QB], sc[i][:], SCH_A, SCH_B,
                    mybir.AluOpType.mult, mybir.AluOpType.add,
                )
                halves.append(et_i[:, i * QB:(i + 1) * QB].bitcast(BF16))
        else:
            et_b = exp_pool.tile([128, 2 * QB], BF16, name="et", tag="et")
            for i in range(2):
                nc.scalar.activation(
                    et_b[:, i * QB:(i + 1) * QB], sc[i][:],
                    mybir.ActivationFunctionType.Exp, scale=scale,
                )
                halves.append(et_b[:, i * QB:(i + 1) * QB])
        return halves

    def emit_av(p, tk, halves):
        g, tb = phase_gtb(p)
        for i in range(2):
            hh = 2 * g + i
            lhsT_v = v_sb[:, tk * HL * DE + hh * DE: tk * HL * DE + (hh + 1) * DE]
            mm(
                ypairs[p][i][0:DE, :],
                lhsT_v,
                halves[i],
                start=(tk == 0),
                stop=(tk == TT - 1),
            )

    norm_state = {}

    # Reciprocal off the DVE critical path: exponent-flip bit-trick seed on
    # the DVE (one cheap int op: 0x7EF311C7 - bits(x) == ((bits(x) XOR
    # 0x7FFFFFFF) - 0x010CEE38) for positive x), then one Newton-Raphson
    # round on the otherwise-idle GpSimd engine. Junk rows of dens2 stay 1.0
    # so no NaN/Inf can leak through the broadcast matmul. Seed max error
    # ~5%; after one NR round ~0.3% worst-case on the denominator.
    def emit_gp_recip(p):
        st = norm_state[p]
        seed = small_pool.tile([128, QB], I32, name="gpr")
        t0 = small_pool.tile([128, QB], F32, name="gpr")
        t1 = small_pool.tile([128, QB], F32R, name="gpr")
        nc.vector.tensor_scalar(
            seed[:], dens2[:].bitcast(I32), -1, 0x7EF311C7,
            mybir.AluOpType.mult, mybir.AluOpType.add,
        )
        r = seed[:].bitcast(F32)
        nc.gpsimd.tensor_tensor(t0[:], dens2[:].bitcast(F32), r, mybir.AluOpType.mult)
        nc.gpsimd.tensor_scalar(
            t0[:], t0[:], -1.0, 2.0, mybir.AluOpType.mult, mybir.AluOpType.add
        )
        with nc.allow_low_precision(reason="f32r moving operand for bc matmul"):
            nc.gpsimd.tensor_tensor(t1[:], r, t0[:], mybir.AluOpType.mult)
        st["rec_pre"] = t1[:]

    def emit_norm_step(p, step):
        """Normalization of phase p: park dens (DVE), GpSimd reciprocal,
        broadcast matmul of the reciprocals (PE), then yT multiplies (DVE)."""
        g, tb = phase_gtb(p)
        yp = ypairs[p]
        if step == 0:
            nc.vector.tensor_copy(dens2[0:1, :], yp[0][D:DE, :])
            nc.vector.tensor_copy(dens2[64:65, :], yp[1][D:DE, :])
            norm_state[p] = {}
        elif step == 1:
            emit_gp_recip(p)
        elif step == 2:
            bc = main_ps.tile([128, QB], F32, name="bc", tag="m")
            mm(bc[:], sel_sb[:], norm_state[p]["rec_pre"], start=True, stop=True)
            rec = small_pool.tile([128, QB], F32R, name="gpr")
            with nc.allow_low_precision(reason="f32r rounding for DVE mul"):
                nc.vector.tensor_copy(rec[:], bc[:])
            norm_state[p]["rec"] = rec
        else:
            rec = norm_state[p]["rec"]
            ycol = g * T + tb * QB
            for i in range(2):
                p0 = i * 64
                nc.vector.tensor_mul(
                    yT_sb[p0:p0 + 64, ycol: ycol + QB],
                    yp[i][0:D, :],
                    rec[p0:p0 + 64, :],
                )
            del norm_state[p]

    NORM_SLOTS = (0, 1, 7, 8)

    def emit_cproj_step(tb, j):
        """c_proj output tile tt = tb*4 + j: two 512-wide halves, then bf16
        staging copies (split ACT/DVE) and one DMA out."""
        tt = tb * (QB // 128) + j
        OSUB = 512
        osb = out_pool.tile([128, C], BF16, name="osb")
        # kk-outer: both 512-wide output halves consume each loaded yT
        # stationary back-to-back, so the second LDWEIGHTS is a cheap reload
        # of the same weights and the PE stream stays at ~216ns/MM.
        opss = [main_ps.tile([128, OSUB], F32, name="ops", tag="m") for _ in range(2)]
        for kk in range(KK):
            lhsT_y = yT_sb[:, kk * T + tt * 128: kk * T + (tt + 1) * 128]
            for ob in range(2):
                mm(
                    opss[ob][:],
                    lhsT_y,
                    wp_sb[:, kk * C + ob * OSUB: kk * C + (ob + 1) * OSUB],
                    start=(kk == 0),
                    stop=(kk == KK - 1),
                )
        nc.scalar.copy(osb[:, 0:OSUB], opss[0][:])
        nc.vector.tensor_copy(osb[:, OSUB:2 * OSUB], opss[1][:])
        nc.sync.dma_start(out[tt * 128:(tt + 1) * 128, :], osb[:])

    # cproj(tb) runs in phase NQB + tb + 1; cproj(NQB-1) lands in the tail.
    CPROJ_SLOTS = (9, 11, 13, 15)

    et_next = None
    sc_next = None
    for p in range(NPH):
        g, tb = phase_gtb(p)
        ypairs[p] = [y_ps.tile([128, QB], F32, name="yps", tag="y") for _ in range(2)]
        if p == 0:
            sc_next = emit_qk(0, 0)
            et_next = emit_exp(0, sc_next)
        for tk in range(TT):
            et_cur = et_next
            if tk < TT - 1:
                sc_next = emit_qk(p, tk + 1)
                et_next = emit_exp(tk + 1, sc_next)
            elif p < NPH - 1:
                sc_next = emit_qk(p + 1, 0)
                et_next = emit_exp(0, sc_next)
            else:
                et_next = None
            emit_av(p, tk, et_cur)
            for kf, tt in vt_sched.get((p, tk), ()):
                emit_vtrans(kf, tt)
            for n_u, (kind, ft, u) in enumerate(unit_sched.get((p, tk), ())):
                emit_unit(kind, ft, u, copy_dve=(n_u % 2 == 0))
            if p >= 1 and tk in NORM_SLOTS:
                emit_norm_step(p - 1, NORM_SLOTS.index(tk))
            if p >= NQB + 1 and tk in CPROJ_SLOTS:
                emit_cproj_step(p - NQB - 1, CPROJ_SLOTS.index(tk))

    # ---- tail: normalize the last phase, final cproj tile set, with dummy
    # matmuls keeping the PE warm through the GpSimd/DVE-side chain. ----
    emit_norm_step(NPH - 1, 0)
    emit_norm_step(NPH - 1, 1)   # seed + GpSimd Newton round (~4us)
    new_dummy_tile()
    dummy_mm(20)
    emit_norm_step(NPH - 1, 2)   # bc matmul + rec copy
    dummy_mm(4)
    emit_norm_step(NPH - 1, 3)   # yT multiplies (DVE)
    dummy_mm(4)
    for j in range(4):
        emit_cproj_step(NQB - 1, j)
        if j < 3:
            dummy_mm(2)

    stack.close()


def build_nc(T=T_FULL, C=C_FULL):
    nc = bass.Bass("TRN2")
    CT = C // 128
    FTW = CT * 128
    xT = nc.dram_tensor("xT", [C, T], BF16, kind="ExternalInput")
    wqk = nc.dram_tensor("wqk", [128, 4 * FTW], BF16, kind="ExternalInput")
    wv = nc.dram_tensor("wv", [128, 2 * FTW], BF16, kind="ExternalInput")
    wp = nc.dram_tensor("wp", [128, (CLOC // 128) * C], BF16, kind="ExternalInput")
    sel = nc.dram_tensor("sel", [128, 128], F32R, kind="ExternalInput")
    zed = nc.dram_tensor("zed", [128, 512], F32R, kind="ExternalInput")
    ident = nc.dram_tensor("ident", [128, 128], BF16, kind="ExternalInput")
    out = nc.dram_tensor("out", [T, C], BF16, kind="ExternalOutput")
    with tile.TileContext(nc) as tc:
        emit_mha_kernel(tc, out[:], xT[:], wqk[:], wv[:], wp[:], sel[:], zed[:], ident[:], T, C)
    return legalize_waits(nc)


def _sbuf_tiled(w):
    """[K, F] -> [128, (K//128)*F] with per-128-row chunks laid side by side
    (the layout emit_mha_kernel indexes as [p, ct*F + f])."""
    K, F = w.shape
    CT = K // 128
    return np.ascontiguousarray(
        w.reshape(CT, 128, F).transpose(1, 0, 2).reshape(128, CT * F)
    )


def make_in_maps(x, W_attn, W_proj):
    """Host-side shard + layout prep for the 8 cores."""
    bf16 = mybir.dt.np(BF16)
    C = x.shape[2]
    sel = np.zeros((128, 128), np.float32)
    sel[0, 0:64] = 1.0
    sel[64, 64:128] = 1.0
    in_maps = []
    for core in range(N_CORES):
        b, hg = divmod(core, N_CORES // B)
        s0, s1 = hg * CLOC, (hg + 1) * CLOC
        Wq = W_attn[s0:s1, :]
        Wk = W_attn[C + s0:C + s1, :]
        Wv = W_attn[2 * C + s0:2 * C + s1, :]
        wqk_slices = [
            _sbuf_tiled(Wq[0:128, :].T), _sbuf_tiled(Wq[128:256, :].T),
            _sbuf_tiled(Wk[0:128, :].T), _sbuf_tiled(Wk[128:256, :].T),
        ]
        wv_slices = [
            _sbuf_tiled(Wv[0:128, :].T), _sbuf_tiled(Wv[128:256, :].T),
        ]
        in_maps.append({
            "sel": sel,
            "zed": np.ones((128, 512), np.float32),
            "ident": np.eye(128).astype(bf16),
            "xT": np.ascontiguousarray(x[b].T).astype(bf16),
            "wqk": np.concatenate(wqk_slices, axis=1).astype(bf16),
            "wv": np.concatenate(wv_slices, axis=1).astype(bf16),
            "wp": _sbuf_tiled(W_proj[:, s0:s1].T).astype(bf16),
        })
    return in_maps


_CACHED_NC = None


def kernel(x, W_attn, W_proj, b_proj, _trace=False):
    global _CACHED_NC
    x = np.asarray(x, dtype=np.float32)
    W_attn = np.asarray(W_attn, dtype=np.float32)
    W_proj = np.asarray(W_proj, dtype=np.float32)
    b_proj = np.asarray(b_proj, dtype=np.float32)

    if _CACHED_NC is None:
        _CACHED_NC = build_nc(T=x.shape[1], C=x.shape[2])
    nc = _CACHED_NC

    in_maps = make_in_maps(x, W_attn, W_proj)
    res = bass_utils.run_bass_kernel_spmd(
        nc, in_maps, core_ids=list(range(N_CORES)), trace=_trace,
    )
    parts = [np.asarray(r["out"], dtype=np.float32) for r in res.results]
    G = N_CORES // B
    out = np.stack(
        [np.sum(parts[b * G:(b + 1) * G], axis=0) + b_proj for b in range(B)], axis=0
    ).astype(np.float32)
    if _trace:
        return out, res
    return out


if __name__ == "__main__":
    nc = build_nc()
    print("built OK")


# revision 21
# speedup vs baseline: 1.1566x; 1.1566x over previous
"""Trainium2 Bass kernel for nn_MultiHeadAttention (B=2, T=2048, C=1024, H=16).

Sharding (8 cores): data-parallel over batch (2) x tensor-parallel over head
groups (4 groups of 4 heads), Megatron-style. Each core computes

    out_partial = softmax((x_b @ Wq_g.T) (x_b @ Wk_g.T).T / sqrt(d)) @ (x_b @ Wv_g.T) @ Wp_g.T

for its 4 heads; the 4 partials per batch are summed on the host (row-parallel
c_proj unshard) - no device collective needed.

Schedule notes:
  - g-OUTER phase order: the four g=0 (heads 0,1) attention phases run first,
    then the four g=1 phases. Only q01 (DMA-paced, with HAM keep-warm dummy
    matmuls filling the DMA gaps), k01 and v01 are projected up-front; the
    q23/k23/v23 projections are injected into the g=0 phases' tk slots, so
    the exp pipeline starts ~35us earlier and phase-B PE stalls are filled
    with useful projection work.
  - All main-PSUM pool slots are one bank ([128,512] fp32): the score tiles
    are allocated as per-head pairs so the row-tiled QK pair always lands in
    two different banks and the effective sc ring is 4 slots; cproj/unit/bc
    allocations only steal one bank at a time.
  - exp is emitted per head-half ([128,512]) with whole slots alternating
    between ACT and DVE (Schraudolph bf16 bit-trick on the DVE slots).
  - The softmax reciprocal runs on the otherwise-idle GpSimd engine as an
    exponent-flip bit-trick seed + two Newton-Raphson rounds on the packed
    dens2 tile (junk rows are kept at 1.0 so no NaN/Inf can leak through the
    broadcast matmul), freeing ~3.4us/phase of DVE time in the exp path.
  - outputs are written bf16 (host accumulates partials in fp32).
  - dummy matmuls on a junk SBUF tile keep the PE HAM clock-gate at 8/8
    through the DMA-paced head region and the serial tail.
"""

import numpy as np

import concourse.bass as bass
import concourse.mybir as mybir
import concourse.tile as tile
from concourse import bass_utils

F32 = mybir.dt.float32
F32R = mybir.dt.float32r
BF16 = mybir.dt.bfloat16
I16 = mybir.dt.int16
I32 = mybir.dt.int32


def legalize_waits(nc, max_waits=1):
    """Walrus codegen in this toolchain rejects instructions carrying more
    than one sync wait. Split extra waits into preceding same-engine NoOps
    at the BIR-JSON level and pin the serialized module on the nc object."""
    import json as _json
    d = _json.loads(nc.to_json_bytes())
    ctr = 0
    for fn in d.get("functions", []):
        for blk in fn.get("blocks", []) or []:
            insts = blk.get("instructions")
            if not insts:
                continue
            out = []
            for inst in insts:
                si = inst.get("sync_info")
                waits = (si or {}).get("on_wait") or []
                if len(waits) > max_waits:
                    keep, extra = waits[:max_waits], waits[max_waits:]
                    for w in extra:
                        ctr += 1
                        out.append({
                            "debug": inst.get("debug", 0),
                            "engine": inst["engine"],
                            "ins": [],
                            "outs": [],
                            "name": f"I-wsplit-{ctr}",
                            "opcode": "NoOp",
                            "sync_info": {"on_wait": [w], "on_update": []},
                        })
                    si["on_wait"] = keep
                out.append(inst)
            blk["instructions"] = out
    raw = _json.dumps(d).encode()
    nc.to_json_bytes = lambda: raw
    return nc

# Problem constants
B, T_FULL, C_FULL = 2, 2048, 1024
H_GLOBAL = 16
D = 64  # head dim
N_CORES = 8
HL = 4  # heads per core
CLOC = HL * D  # 256 local channels


def emit_mha_kernel(tc, out, xT, wqk, wv, wp, sel, zed, ident, T, C):
    """Emit the per-core MHA kernel into TileContext tc.

    out: dram [T, C] bf16 (partial output)
    xT:  dram [C, T]
    wqk: dram [128, 4*CT*128]  (ft-major: q01, q23, k01, k23)
    wv:  dram [128, 2*CT*128]  (ft-major: v01, v23)
    wp:  dram [128, KK*C]
    sel: dram [128, 128] (bc selector); zed: dram [128,512] of ONES
    """
    nc = tc.nc
    sel = sel.bitcast(F32R)
    zed = zed.bitcast(F32R)

    CT = C // 128          # c (contraction) tiles for projections
    TT = T // 128          # t tiles
    KK = CLOC // 128       # c_loc tiles (2)
    SUB = 512              # projection unit free width (one PSUM bank fp32)
    NU = T // SUB          # projection units per f-tile (4)
    DE = D + 1             # v columns incl. ones
    QB = min(512, T)       # tq block per head in phase B
    NQB = T // QB
    NPH = NQB * (HL // 2)  # attention phases: 8
    FTW = CT * 128         # columns per ft slice of wqk/wv (1024)

    import contextlib
    stack = contextlib.ExitStack()

    persist = stack.enter_context(tc.tile_pool(name="persist", bufs=1))
    main_ps = stack.enter_context(tc.tile_pool(name="main_ps", bufs=4, space="PSUM"))
    y_ps = stack.enter_context(tc.tile_pool(name="y_ps", bufs=4, space="PSUM"))
    exp_pool = stack.enter_context(tc.tile_pool(name="exp_pool", bufs=4))
    out_pool = stack.enter_context(tc.tile_pool(name="out_pool", bufs=2))
    small_pool = stack.enter_context(tc.tile_pool(name="small_pool", bufs=3))

    # ---- persistent SBUF tensors ----
    xt_sb = persist.tile([128, CT * T], BF16, name="xt_sb")
    wqk_sb = persist.tile([128, 4 * FTW], BF16, name="wqk_sb")
    wv_sb = persist.tile([128, 2 * FTW], BF16, name="wv_sb")
    wp_sb = persist.tile([128, KK * C], BF16, name="wp_sb")
    qk_sb = persist.tile([128, 4 * T], BF16, name="qk_sb")
    v_sb = persist.tile([128, TT * HL * DE], BF16, name="v_sb")
    vt_sb = persist.tile([128, 2 * T], BF16, name="vt_sb")
    yT_sb = persist.tile([128, KK * T], BF16, name="yT_sb")
    sel_sb = persist.tile([128, 128], F32R, name="sel_sb")
    dens2 = persist.tile([128, QB], F32R, name="dens2")
    ident_sb = persist.tile([128, 128], BF16, name="ident_sb")
    junk_sb = persist.tile([128, 512], BF16, name="junk_sb")

    def mm(out_ap, lhsT, rhs, **kw):
        nc.tensor.matmul(out_ap, lhsT, rhs, **kw)

    # ---- warm up the ACT exp table during the input DMA window ----
    warm = small_pool.tile([128, 1], F32, name="warm")
    const0 = nc.const_aps.aps[(mybir.dt.float32, 0.0)]
    nc.scalar.activation(warm[:], const0, mybir.ActivationFunctionType.Exp)

    # junk tile for HAM keep-warm dummy matmuls (no DMA dependency)
    nc.vector.memset(junk_sb[:], 0.0)

    dummy_state = {"tile": None}

    def new_dummy_tile():
        dummy_state["tile"] = main_ps.tile([128, 512], F32, name="dmy", tag="m")

    def dummy_mm(n=1):
        dp = dummy_state["tile"]
        for _ in range(n):
            mm(dp[:], junk_sb[:, 0:128], junk_sb[:, 0:512], start=True, stop=True)

    # ---- input DMAs: small tables first, then the critical prefix
    # (wqk ft0 -> xT -> wqk ft2 -> wv ft0), then the deferred weights. ----
    nc.sync.dma_start(ident_sb[:], ident[:])
    nc.sync.dma_start(sel_sb[:], sel[:])
    nc.sync.dma_start(dens2[:], zed[:])
    nc.vector.memset(
        v_sb[:].rearrange("p (t h e) -> p t h e", t=TT, h=HL)[:, :, :, D:DE], 1.0
    )
    nc.sync.dma_start(wqk_sb[:, 0:FTW], wqk[:, 0:FTW])  # q01
    for ct in range(CT):
        nc.sync.dma_start(
            xt_sb[:, ct * T:(ct + 1) * T], xT[ct * 128:(ct + 1) * 128, :]
        )
    nc.sync.dma_start(wqk_sb[:, 2 * FTW:3 * FTW], wqk[:, 2 * FTW:3 * FTW])  # k01
    nc.sync.dma_start(wv_sb[:, 0:FTW], wv[:, 0:FTW])  # v01
    nc.sync.dma_start(wqk_sb[:, FTW:2 * FTW], wqk[:, FTW:2 * FTW])  # q23
    nc.sync.dma_start(wqk_sb[:, 3 * FTW:4 * FTW], wqk[:, 3 * FTW:4 * FTW])  # k23
    nc.sync.dma_start(wv_sb[:, FTW:2 * FTW], wv[:, FTW:2 * FTW])  # v23
    nc.sync.dma_start(wp_sb[:], wp[:])

    # ---- A-head ----
    def w_lhsT(kind, ft, ct):
        if kind == "qk":
            return wqk_sb[:, ft * FTW + ct * 128: ft * FTW + (ct + 1) * 128]
        return wv_sb[:, ft * FTW + ct * 128: ft * FTW + (ct + 1) * 128]

    def proj_dst(kind, ft):
        if kind == "qk":
            return qk_sb, ft * T
        return vt_sb, ft * T

    new_dummy_tile()
    dummy_mm(16)  # ~3.4us of PE busy to flip HAM to 8/8 during the DMA lead-in

    # q01 = ft 0 of wqk: ct-outer so matmuls stream behind the xT DMA.
    a_groups = [y_ps.tile([128, 512], F32, name="ahead", tag="y") for _ in range(NU)]
    for ct in range(CT):
        if 1 <= ct <= 3:
            dummy_mm(4)
        for u in range(NU):
            mm(
                a_groups[u][:],
                w_lhsT("qk", 0, ct),
                xt_sb[:, ct * T + u * SUB: ct * T + (u + 1) * SUB],
                start=(ct == 0),
                stop=(ct == CT - 1),
            )
    dst, col = proj_dst("qk", 0)
    for u in range(NU):
        nc.vector.tensor_copy(dst[:, col + u * SUB: col + (u + 1) * SUB], a_groups[u][:])

    def emit_proj_ftile(kind, ft):
        """One full f-tile projection (4 units x 8 ct), PE-bound, via y_ps."""
        groups = [y_ps.tile([128, 512], F32, name="ahead", tag="y") for _ in range(NU)]
        for ct in range(CT):
            for u in range(NU):
                mm(
                    groups[u][:],
                    w_lhsT(kind, ft, ct),
                    xt_sb[:, ct * T + u * SUB: ct * T + (u + 1) * SUB],
                    start=(ct == 0),
                    stop=(ct == CT - 1),
                )
        dst, col = proj_dst(kind, ft)
        for u in range(NU):
            nc.vector.tensor_copy(
                dst[:, col + u * SUB: col + (u + 1) * SUB], groups[u][:]
            )

    emit_proj_ftile("qk", 2)  # k01
    emit_proj_ftile("v", 0)   # v01

    def emit_unit(kind, ft, u, copy_dve):
        """One projection unit (8 ct matmuls into one PSUM bank + copy),
        injected into a phase-B tk slot."""
        ps = main_ps.tile([128, 512], F32, name="unit", tag="m")
        for ct in range(CT):
            mm(
                ps[:],
                w_lhsT(kind, ft, ct),
                xt_sb[:, ct * T + u * SUB: ct * T + (u + 1) * SUB],
                start=(ct == 0),
                stop=(ct == CT - 1),
            )
        dst, col = proj_dst(kind, ft)
        dap = dst[:, col + u * SUB: col + (u + 1) * SUB]
        if copy_dve:
            nc.vector.tensor_copy(dap, ps[:])
        else:
            nc.scalar.copy(dap, ps[:])

    # A-leftover unit schedule: (phase, slot) -> list of (kind, ft, u)
    unit_sched = {}
    for i in range(NU):
        unit_sched.setdefault((1, 3 if i < 2 else 11), []).append(("qk", 1, i))
        unit_sched.setdefault((2, 3 if i < 2 else 11), []).append(("qk", 3, i))
        unit_sched.setdefault((3, 3 if i < 2 else 11), []).append(("v", 1, i))

    def emit_vtrans(kf, tt):
        """PE-transpose one 128x128 tile of vt_sb (half kf) into v_sb."""
        tp = main_ps.tile([128, 128], BF16, name="tp", tag="m")
        nc.tensor.transpose(
            tp[:, 0:128],
            vt_sb[:, kf * T + tt * 128: kf * T + (tt + 1) * 128],
            ident_sb[:],
        )
        for hh in range(2):
            h = kf * 2 + hh
            nc.vector.tensor_copy(
                v_sb[:, tt * HL * DE + h * DE: tt * HL * DE + h * DE + D],
                tp[:, hh * 64: hh * 64 + 64],
            )

    # vtrans schedule: (phase, slot) -> list of (kf, tt). Tiles are emitted
    # (after that slot's AV) at least two slots before their first AV read.
    vt_sched = {}
    emit_vtrans(0, 0)
    emit_vtrans(0, 1)
    for s in range(0, 14, 2):   # phase 0, kf=0: tiles s+2, s+3 at slot s
        vt_sched[(0, s)] = [(0, s + 2), (0, s + 3)]
    vt_sched[(3, 12)] = [(1, 0), (1, 1)]
    vt_sched[(3, 14)] = [(1, 2), (1, 3)]
    for s in range(0, 12, 2):   # phase NQB, kf=1: tiles s+4, s+5 at slot s
        vt_sched[(NQB, s)] = [(1, s + 4), (1, s + 5)]

    # ---- Phase B: attention. Phase p: g = p // NQB, tb = p % NQB. ----
    ypairs = [None] * NPH
    scale = 1.0 / np.sqrt(D)

    def phase_gtb(p):
        return p // NQB, p % NQB

    def emit_qk(p, tk):
        g, tb = phase_gtb(p)
        qcol = g * T
        kcol = (2 + g) * T
        sc = []
        for i in range(2):
            p0 = i * 64
            sch = main_ps.tile([128, QB], F32, name="sc", tag="m")
            mm(
                sch[:],
                qk_sb[p0:p0 + 64, kcol + tk * 128: kcol + (tk + 1) * 128],
                qk_sb[p0:p0 + 64, qcol + tb * QB: qcol + (tb + 1) * QB],
                start=True,
                stop=True,
            )
            sc.append(sch)
        return sc

    # Schraudolph bf16 exp on the DVE for some slots per phase: bf16 bits of
    # exp(scale*s) ~= round(s*(scale*128/ln2) + (127*128 - 7.42)). The uniform
    # half-bit rounding bias cancels between numerator and denominator of the
    # softmax; residual noise is ~1.5% rms on the offloaded tiles.
    SCH_A = float(scale * 128.0 / np.log(2.0))
    SCH_B = float(127.0 * 128.0 - 7.42 + 0.5)
    DVE_SLOTS = (1, 4, 6, 9, 12, 15)

    def emit_exp(tk, sc):
        """exp of slot tk's scores, emitted one slot EARLY so the AV pair
        never head-of-line blocks the PE queue (keeps LDWEIGHTS hidden)."""
        halves = []
        if tk in DVE_SLOTS:
            et_i = exp_pool.tile([128, 2 * QB], I16, name="et", tag="et")
            for i in range(2):
                nc.vector.tensor_scalar(
                    et_i[:, i * QB:(i + 1) * QB], sc[i][:], SCH_A, SCH_B,
                    mybir.AluOpType.mult, mybir.AluOpType.add,
                )
                halves.append(et_i[:, i * QB:(i + 1) * QB].bitcast(BF16))
        else:
            et_b = exp_pool.tile([128, 2 * QB], BF16, name="et", tag="et")
            for i in range(2):
                nc.scalar.activation(
                    et_b[:, i * QB:(i + 1) * QB], sc[i][:],
                    mybir.ActivationFunctionType.Exp, scale=scale,
                )
                halves.append(et_b[:, i * QB:(i + 1) * QB])
        return halves

    def emit_av(p, tk, halves):
        g, tb = phase_gtb(p)
        for i in range(2):
            hh = 2 * g + i
            lhsT_v = v_sb[:, tk * HL * DE + hh * DE: tk * HL * DE + (hh + 1) * DE]
            mm(
                ypairs[p][i][0:DE, :],
                lhsT_v,
                halves[i],
                start=(tk == 0),
                stop=(tk == TT - 1),
            )

    norm_state = {}

    # Reciprocal off the DVE critical path: exponent-flip bit-trick seed on
    # the DVE (one cheap int op: 0x7EF311C7 - bits(x) == ((bits(x) XOR
    # 0x7FFFFFFF) - 0x010CEE38) for positive x), then one Newton-Raphson
    # round on the otherwise-idle GpSimd engine. Junk rows of dens2 stay 1.0
    # so no NaN/Inf can leak through the broadcast matmul. Seed max error
    # ~5%; after one NR round ~0.3% worst-case on the denominator.
    def emit_gp_recip(p):
        st = norm_state[p]
        seed = small_pool.tile([128, QB], I32, name="gpr")
        t0 = small_pool.tile([128, QB], F32, name="gpr")
        t1 = small_pool.tile([128, QB], F32R, name="gpr")
        nc.vector.tensor_scalar(
            seed[:], dens2[:].bitcast(I32), -1, 0x7EF311C7,
            mybir.AluOpType.mult, mybir.AluOpType.add,
        )
        r = seed[:].bitcast(F32)
        nc.gpsimd.tensor_tensor(t0[:], dens2[:].bitcast(F32), r, mybir.AluOpType.mult)
        nc.gpsimd.tensor_scalar(
            t0[:], t0[:], -1.0, 2.0, mybir.AluOpType.mult, mybir.AluOpType.add
        )
        with nc.allow_low_precision(reason="f32r moving operand for bc matmul"):
            nc.gpsimd.tensor_tensor(t1[:], r, t0[:], mybir.AluOpType.mult)
        st["rec_pre"] = t1[:]

    def emit_norm_step(p, step):
        """Normalization of phase p: park dens (DVE), GpSimd reciprocal,
        broadcast matmul of the reciprocals (PE), then yT multiplies (DVE)."""
        g, tb = phase_gtb(p)
        yp = ypairs[p]
        if step == 0:
            nc.vector.tensor_copy(dens2[0:1, :], yp[0][D:DE, :])
            nc.vector.tensor_copy(dens2[64:65, :], yp[1][D:DE, :])
            norm_state[p] = {}
        elif step == 1:
            emit_gp_recip(p)
        elif step == 2:
            bc = main_ps.tile([128, QB], F32, name="bc", tag="m")
            mm(bc[:], sel_sb[:], norm_state[p]["rec_pre"], start=True, stop=True)
            rec = small_pool.tile([128, QB], F32R, name="gpr")
            with nc.allow_low_precision(reason="f32r rounding for DVE mul"):
                nc.vector.tensor_copy(rec[:], bc[:])
            norm_state[p]["rec"] = rec
        else:
            rec = norm_state[p]["rec"]
            ycol = g * T + tb * QB
            for i in range(2):
                p0 = i * 64
                nc.vector.tensor_mul(
                    yT_sb[p0:p0 + 64, ycol: ycol + QB],
                    yp[i][0:D, :],
                    rec[p0:p0 + 64, :],
                )
            del norm_state[p]

    NORM_SLOTS = (0, 1, 7, 8)

    def emit_cproj_step(tb, j):
        """c_proj output tile tt = tb*4 + j: two 512-wide halves, then bf16
        staging copies (split ACT/DVE) and one DMA out."""
        tt = tb * (QB // 128) + j
        OSUB = 512
        osb = out_pool.tile([128, C], BF16, name="osb")
        # kk-outer: both 512-wide output halves consume each loaded yT
        # stationary back-to-back, so the second LDWEIGHTS is a cheap reload
        # of the same weights and the PE stream stays at ~216ns/MM.
        opss = [main_ps.tile([128, OSUB], F32, name="ops", tag="m") for _ in range(2)]
        for kk in range(KK):
            lhsT_y = yT_sb[:, kk * T + tt * 128: kk * T + (tt + 1) * 128]
            for ob in range(2):
                mm(
                    opss[ob][:],
                    lhsT_y,
                    wp_sb[:, kk * C + ob * OSUB: kk * C + (ob + 1) * OSUB],
                    start=(kk == 0),
                    stop=(kk == KK - 1),
                )
        nc.scalar.copy(osb[:, 0:OSUB], opss[0][:])
        nc.vector.tensor_copy(osb[:, OSUB:2 * OSUB], opss[1][:])
        nc.sync.dma_start(out[tt * 128:(tt + 1) * 128, :], osb[:])

    # cproj(tb) runs in phase NQB + tb + 1; cproj(NQB-1) lands in the tail.
    CPROJ_SLOTS = (9, 11, 13, 15)

    et_next = None
    sc_next = None
    for p in range(NPH):
        g, tb = phase_gtb(p)
        ypairs[p] = [y_ps.tile([128, QB], F32, name="yps", tag="y") for _ in range(2)]
        if p == 0:
            sc_next = emit_qk(0, 0)
            et_next = emit_exp(0, sc_next)
        for tk in range(TT):
            et_cur = et_next
            if tk < TT - 1:
                sc_next = emit_qk(p, tk + 1)
                et_next = emit_exp(tk + 1, sc_next)
            elif p < NPH - 1:
                sc_next = emit_qk(p + 1, 0)
                et_next = emit_exp(0, sc_next)
            else:
                et_next = None
            emit_av(p, tk, et_cur)
            for kf, tt in vt_sched.get((p, tk), ()):
                emit_vtrans(kf, tt)
            for n_u, (kind, ft, u) in enumerate(unit_sched.get((p, tk), ())):
                emit_unit(kind, ft, u, copy_dve=(n_u % 2 == 0))
            if p >= 1 and tk in NORM_SLOTS:
                emit_norm_step(p - 1, NORM_SLOTS.index(tk))
            if p >= NQB + 1 and tk in CPROJ_SLOTS:
                emit_cproj_step(p - NQB - 1, CPROJ_SLOTS.index(tk))

    # ---- tail: normalize the last phase, final cproj tile set, with dummy
    # matmuls keeping the PE warm through the GpSimd/DVE-side chain. ----
    emit_norm_step(NPH - 1, 0)
    emit_norm_step(NPH - 1, 1)   # seed + GpSimd Newton round (~4us)
    new_dummy_tile()
    dummy_mm(20)
    emit_norm_step(NPH - 1, 2)   # bc matmul + rec copy
    dummy_mm(4)
    emit_norm_step(NPH - 1, 3)   # yT multiplies (DVE)
    dummy_mm(4)
    for j in range(4):
        emit_cproj_step(NQB - 1, j)
        if j < 3:
            dummy_mm(2)

    stack.close()


def build_nc(T=T_FULL, C=C_FULL):
    nc = bass.Bass("TRN2")
    CT = C // 128
    FTW = CT * 128
    xT = nc.dram_tensor("xT", [C, T], BF16, kind="ExternalInput")
    wqk = nc.dram_tensor("wqk", [128, 4 * FTW], BF16, kind="ExternalInput")
    wv = nc.dram_tensor("wv", [128, 2 * FTW], BF16, kind="ExternalInput")
    wp = nc.dram_tensor("wp", [128, (CLOC // 128) * C], BF16, kind="ExternalInput")
    sel = nc.dram_tensor("sel", [128, 128], F32R, kind="ExternalInput")
    zed = nc.dram_tensor("zed", [128, 512], F32R, kind="ExternalInput")
    ident = nc.dram_tensor("ident", [128, 128], BF16, kind="ExternalInput")
    out = nc.dram_tensor("out", [T, C], BF16, kind="ExternalOutput")
    with tile.TileContext(nc) as tc:
        emit_mha_kernel(tc, out[:], xT[:], wqk[:], wv[:], wp[:], sel[:], zed[:], ident[:], T, C)
    return legalize_waits(nc)


def _sbuf_tiled(w):
    """[K, F] -> [128, (K//128)*F] with per-128-row chunks laid side by side
    (the layout emit_mha_kernel indexes as [p, ct*F + f])."""
    K, F = w.shape
    CT = K // 128
    return np.ascontiguousarray(
        w.reshape(CT, 128, F).transpose(1, 0, 2).reshape(128, CT * F)
    )


def make_in_maps(x, W_attn, W_proj):
    """Host-side shard + layout prep for the 8 cores."""
    bf16 = mybir.dt.np(BF16)
    C = x.shape[2]
    sel = np.zeros((128, 128), np.float32)
    sel[0, 0:64] = 1.0
    sel[64, 64:128] = 1.0
    in_maps = []
    for core in range(N_CORES):
        b, hg = divmod(core, N_CORES // B)
        s0, s1 = hg * CLOC, (hg + 1) * CLOC
        Wq = W_attn[s0:s1, :]
        Wk = W_attn[C + s0:C + s1, :]
        Wv = W_attn[2 * C + s0:2 * C + s1, :]
        wqk_slices = [
            _sbuf_tiled(Wq[0:128, :].T), _sbuf_tiled(Wq[128:256, :].T),
            _sbuf_tiled(Wk[0:128, :].T), _sbuf_tiled(Wk[128:256, :].T),
        ]
        wv_slices = [
            _sbuf_tiled(Wv[0:128, :].T), _sbuf_tiled(Wv[128:256, :].T),
        ]
        in_maps.append({
            "sel": sel,
            "zed": np.ones((128, 512), np.float32),
            "ident": np.eye(128).astype(bf16),
            "xT": np.ascontiguousarray(x[b].T).astype(bf16),
            "wqk": np.concatenate(wqk_slices, axis=1).astype(bf16),
            "wv": np.concatenate(wv_slices, axis=1).astype(bf16),
            "wp": _sbuf_tiled(W_proj[:, s0:s1].T).astype(bf16),
        })
    return in_maps


_CACHED_NC = None


def kernel(x, W_attn, W_proj, b_proj, _trace=False):
    global _CACHED_NC
    x = np.asarray(x, dtype=np.float32)
    W_attn = np.asarray(W_attn, dtype=np.float32)
    W_proj = np.asarray(W_proj, dtype=np.float32)
    b_proj = np.asarray(b_proj, dtype=np.float32)

    if _CACHED_NC is None:
        _CACHED_NC = build_nc(T=x.shape[1], C=x.shape[2])
    nc = _CACHED_NC

    in_maps = make_in_maps(x, W_attn, W_proj)
    res = bass_utils.run_bass_kernel_spmd(
        nc, in_maps, core_ids=list(range(N_CORES)), trace=_trace,
    )
    parts = [np.asarray(r["out"], dtype=np.float32) for r in res.results]
    G = N_CORES // B
    out = np.stack(
        [np.sum(parts[b * G:(b + 1) * G], axis=0) + b_proj for b in range(B)], axis=0
    ).astype(np.float32)
    if _trace:
        return out, res
    return out


if __name__ == "__main__":
    nc = build_nc()
    print("built OK")


# revision 22
# speedup vs baseline: 1.1733x; 1.0144x over previous
"""Trainium2 Bass kernel for nn_MultiHeadAttention (B=2, T=2048, C=1024, H=16).

Sharding (8 cores): data-parallel over batch (2) x tensor-parallel over head
groups (4 groups of 4 heads), Megatron-style. Each core computes

    out_partial = softmax((x_b @ Wq_g.T) (x_b @ Wk_g.T).T / sqrt(d)) @ (x_b @ Wv_g.T) @ Wp_g.T

for its 4 heads; the 4 partials per batch are summed on the host (row-parallel
c_proj unshard) - no device collective needed.

Schedule notes:
  - g-OUTER phase order: the four g=0 (heads 0,1) attention phases run first,
    then the four g=1 phases. Only q01 (DMA-paced, with HAM keep-warm dummy
    matmuls filling the DMA gaps), k01 and v01 are projected up-front; the
    q23/k23/v23 projections are injected into the g=0 phases' tk slots, so
    the exp pipeline starts ~35us earlier and phase-B PE stalls are filled
    with useful projection work.
  - All main-PSUM pool slots are one bank ([128,512] fp32): the score tiles
    are allocated as per-head pairs so the row-tiled QK pair always lands in
    two different banks and the effective sc ring is 4 slots; cproj/unit/bc
    allocations only steal one bank at a time.
  - exp is emitted per head-half ([128,512]) with whole slots alternating
    between ACT and DVE (Schraudolph bf16 bit-trick on the DVE slots).
  - The softmax reciprocal runs on the otherwise-idle GpSimd engine as an
    exponent-flip bit-trick seed + two Newton-Raphson rounds on the packed
    dens2 tile (junk rows are kept at 1.0 so no NaN/Inf can leak through the
    broadcast matmul), freeing ~3.4us/phase of DVE time in the exp path.
  - outputs are written bf16 (host accumulates partials in fp32).
  - dummy matmuls on a junk SBUF tile keep the PE HAM clock-gate at 8/8
    through the DMA-paced head region and the serial tail.
"""

import numpy as np

import concourse.bass as bass
import concourse.mybir as mybir
import concourse.tile as tile
from concourse import bass_utils

F32 = mybir.dt.float32
F32R = mybir.dt.float32r
BF16 = mybir.dt.bfloat16
I16 = mybir.dt.int16
I32 = mybir.dt.int32


def legalize_waits(nc, max_waits=1):
    """Walrus codegen in this toolchain rejects instructions carrying more
    than one sync wait. Split extra waits into preceding same-engine NoOps
    at the BIR-JSON level and pin the serialized module on the nc object."""
    import json as _json
    d = _json.loads(nc.to_json_bytes())
    ctr = 0
    for fn in d.get("functions", []):
        for blk in fn.get("blocks", []) or []:
            insts = blk.get("instructions")
            if not insts:
                continue
            out = []
            for inst in insts:
                si = inst.get("sync_info")
                waits = (si or {}).get("on_wait") or []
                if len(waits) > max_waits:
                    keep, extra = waits[:max_waits], waits[max_waits:]
                    for w in extra:
                        ctr += 1
                        out.append({
                            "debug": inst.get("debug", 0),
                            "engine": inst["engine"],
                            "ins": [],
                            "outs": [],
                            "name": f"I-wsplit-{ctr}",
                            "opcode": "NoOp",
                            "sync_info": {"on_wait": [w], "on_update": []},
                        })
                    si["on_wait"] = keep
                out.append(inst)
            blk["instructions"] = out
    raw = _json.dumps(d).encode()
    nc.to_json_bytes = lambda: raw
    return nc

# Problem constants
B, T_FULL, C_FULL = 2, 2048, 1024
H_GLOBAL = 16
D = 64  # head dim
N_CORES = 8
HL = 4  # heads per core
CLOC = HL * D  # 256 local channels


def emit_mha_kernel(tc, out, xT, wqk, wv, wp, sel, zed, ident, T, C):
    """Emit the per-core MHA kernel into TileContext tc.

    out: dram [T, C] bf16 (partial output)
    xT:  dram [C, T]
    wqk: dram [128, 4*CT*128]  (ft-major: q01, q23, k01, k23)
    wv:  dram [128, 2*CT*128]  (ft-major: v01, v23)
    wp:  dram [128, KK*C]
    sel: dram [128, 128] (bc selector); zed: dram [128,512] of ONES
    """
    nc = tc.nc
    sel = sel.bitcast(F32R)
    zed = zed.bitcast(F32R)

    CT = C // 128          # c (contraction) tiles for projections
    TT = T // 128          # t tiles
    KK = CLOC // 128       # c_loc tiles (2)
    SUB = 512              # projection unit free width (one PSUM bank fp32)
    NU = T // SUB          # projection units per f-tile (4)
    DE = D + 1             # v columns incl. ones
    QB = min(512, T)       # tq block per head in phase B
    NQB = T // QB
    NPH = NQB * (HL // 2)  # attention phases: 8
    FTW = CT * 128         # columns per ft slice of wqk/wv (1024)

    import contextlib
    stack = contextlib.ExitStack()

    persist = stack.enter_context(tc.tile_pool(name="persist", bufs=1))
    main_ps = stack.enter_context(tc.tile_pool(name="main_ps", bufs=4, space="PSUM"))
    y_ps = stack.enter_context(tc.tile_pool(name="y_ps", bufs=4, space="PSUM"))
    exp_pool = stack.enter_context(tc.tile_pool(name="exp_pool", bufs=4))
    out_pool = stack.enter_context(tc.tile_pool(name="out_pool", bufs=2))
    small_pool = stack.enter_context(tc.tile_pool(name="small_pool", bufs=3))

    # ---- persistent SBUF tensors ----
    xt_sb = persist.tile([128, CT * T], BF16, name="xt_sb")
    wqk_sb = persist.tile([128, 4 * FTW], BF16, name="wqk_sb")
    wv_sb = persist.tile([128, 2 * FTW], BF16, name="wv_sb")
    wp_sb = persist.tile([128, KK * C], BF16, name="wp_sb")
    qk_sb = persist.tile([128, 4 * T], BF16, name="qk_sb")
    v_sb = persist.tile([128, TT * HL * DE], BF16, name="v_sb")
    vt_sb = persist.tile([128, 2 * T], BF16, name="vt_sb")
    yT_sb = persist.tile([128, KK * T], BF16, name="yT_sb")
    sel_sb = persist.tile([128, 128], F32R, name="sel_sb")
    dens2 = persist.tile([128, QB], F32R, name="dens2")
    ident_sb = persist.tile([128, 128], BF16, name="ident_sb")
    junk_sb = persist.tile([128, 512], BF16, name="junk_sb")

    def mm(out_ap, lhsT, rhs, **kw):
        nc.tensor.matmul(out_ap, lhsT, rhs, **kw)

    # ---- warm up the ACT exp table during the input DMA window ----
    warm = small_pool.tile([128, 1], F32, name="warm")
    const0 = nc.const_aps.aps[(mybir.dt.float32, 0.0)]
    nc.scalar.activation(warm[:], const0, mybir.ActivationFunctionType.Exp)

    # junk tile for HAM keep-warm dummy matmuls (no DMA dependency)
    nc.vector.memset(junk_sb[:], 0.0)

    dummy_state = {"tile": None}

    def new_dummy_tile():
        dummy_state["tile"] = main_ps.tile([128, 512], F32, name="dmy", tag="m")

    def dummy_mm(n=1):
        dp = dummy_state["tile"]
        for _ in range(n):
            mm(dp[:], junk_sb[:, 0:128], junk_sb[:, 0:512], start=True, stop=True)

    # ---- input DMAs: small tables first, then the critical prefix
    # (wqk ft0 -> xT -> wqk ft2 -> wv ft0), then the deferred weights. ----
    nc.sync.dma_start(ident_sb[:], ident[:])
    nc.sync.dma_start(sel_sb[:], sel[:])
    nc.sync.dma_start(dens2[:], zed[:])
    nc.vector.memset(
        v_sb[:].rearrange("p (t h e) -> p t h e", t=TT, h=HL)[:, :, :, D:DE], 1.0
    )
    nc.sync.dma_start(wqk_sb[:, 0:FTW], wqk[:, 0:FTW])  # q01
    for ct in range(CT):
        nc.sync.dma_start(
            xt_sb[:, ct * T:(ct + 1) * T], xT[ct * 128:(ct + 1) * 128, :]
        )
    nc.sync.dma_start(wqk_sb[:, 2 * FTW:3 * FTW], wqk[:, 2 * FTW:3 * FTW])  # k01
    nc.sync.dma_start(wv_sb[:, 0:FTW], wv[:, 0:FTW])  # v01
    nc.sync.dma_start(wqk_sb[:, FTW:2 * FTW], wqk[:, FTW:2 * FTW])  # q23
    nc.sync.dma_start(wqk_sb[:, 3 * FTW:4 * FTW], wqk[:, 3 * FTW:4 * FTW])  # k23
    nc.sync.dma_start(wv_sb[:, FTW:2 * FTW], wv[:, FTW:2 * FTW])  # v23
    nc.sync.dma_start(wp_sb[:], wp[:])

    # ---- A-head ----
    def w_lhsT(kind, ft, ct):
        if kind == "qk":
            return wqk_sb[:, ft * FTW + ct * 128: ft * FTW + (ct + 1) * 128]
        return wv_sb[:, ft * FTW + ct * 128: ft * FTW + (ct + 1) * 128]

    def proj_dst(kind, ft):
        if kind == "qk":
            return qk_sb, ft * T
        return vt_sb, ft * T

    new_dummy_tile()
    dummy_mm(16)  # ~3.4us of PE busy to flip HAM to 8/8 during the DMA lead-in

    # q01 = ft 0 of wqk: ct-outer so matmuls stream behind the xT DMA.
    a_groups = [y_ps.tile([128, 512], F32, name="ahead", tag="y") for _ in range(NU)]
    for ct in range(CT):
        if 1 <= ct <= 3:
            dummy_mm(4)
        for u in range(NU):
            mm(
                a_groups[u][:],
                w_lhsT("qk", 0, ct),
                xt_sb[:, ct * T + u * SUB: ct * T + (u + 1) * SUB],
                start=(ct == 0),
                stop=(ct == CT - 1),
            )
    dst, col = proj_dst("qk", 0)
    for u in range(NU):
        nc.vector.tensor_copy(dst[:, col + u * SUB: col + (u + 1) * SUB], a_groups[u][:])

    def emit_proj_ftile(kind, ft):
        """One full f-tile projection (4 units x 8 ct), PE-bound, via y_ps."""
        groups = [y_ps.tile([128, 512], F32, name="ahead", tag="y") for _ in range(NU)]
        for ct in range(CT):
            for u in range(NU):
                mm(
                    groups[u][:],
                    w_lhsT(kind, ft, ct),
                    xt_sb[:, ct * T + u * SUB: ct * T + (u + 1) * SUB],
                    start=(ct == 0),
                    stop=(ct == CT - 1),
                )
        dst, col = proj_dst(kind, ft)
        for u in range(NU):
            nc.vector.tensor_copy(
                dst[:, col + u * SUB: col + (u + 1) * SUB], groups[u][:]
            )

    emit_proj_ftile("qk", 2)  # k01
    emit_proj_ftile("v", 0)   # v01

    def emit_unit(kind, ft, u, copy_dve):
        """One projection unit (8 ct matmuls into one PSUM bank + copy),
        injected into a phase-B tk slot."""
        ps = main_ps.tile([128, 512], F32, name="unit", tag="m")
        for ct in range(CT):
            mm(
                ps[:],
                w_lhsT(kind, ft, ct),
                xt_sb[:, ct * T + u * SUB: ct * T + (u + 1) * SUB],
                start=(ct == 0),
                stop=(ct == CT - 1),
            )
        dst, col = proj_dst(kind, ft)
        dap = dst[:, col + u * SUB: col + (u + 1) * SUB]
        if copy_dve:
            nc.vector.tensor_copy(dap, ps[:])
        else:
            nc.scalar.copy(dap, ps[:])

    # A-leftover unit schedule: (phase, slot) -> list of (kind, ft, u).
    # One unit per slot: a burst of two held 2 of the 4 main-ring slots for
    # ~3.5us and starved the score-tile ring; spread keeps the ring deep.
    UNIT_SLOTS = (2, 6, 10, 14)
    unit_sched = {}
    for i in range(NU):
        unit_sched.setdefault((1, UNIT_SLOTS[i]), []).append(("qk", 1, i))
        unit_sched.setdefault((2, UNIT_SLOTS[i]), []).append(("qk", 3, i))
        unit_sched.setdefault((3, UNIT_SLOTS[i]), []).append(("v", 1, i))

    def emit_vtrans(kf, tt):
        """PE-transpose one 128x128 tile of vt_sb (half kf) into v_sb."""
        tp = main_ps.tile([128, 128], BF16, name="tp", tag="m")
        nc.tensor.transpose(
            tp[:, 0:128],
            vt_sb[:, kf * T + tt * 128: kf * T + (tt + 1) * 128],
            ident_sb[:],
        )
        for hh in range(2):
            h = kf * 2 + hh
            nc.vector.tensor_copy(
                v_sb[:, tt * HL * DE + h * DE: tt * HL * DE + h * DE + D],
                tp[:, hh * 64: hh * 64 + 64],
            )

    # vtrans schedule: (phase, slot) -> list of (kf, tt). Tiles are emitted
    # (after that slot's AV) at least two slots before their first AV read.
    vt_sched = {}
    emit_vtrans(0, 0)
    emit_vtrans(0, 1)
    for s in range(0, 14, 2):   # phase 0, kf=0: tiles s+2, s+3 at slot s
        vt_sched[(0, s)] = [(0, s + 2), (0, s + 3)]
    vt_sched[(3, 12)] = [(1, 0), (1, 1)]
    vt_sched[(3, 14)] = [(1, 2), (1, 3)]
    for s in range(0, 12, 2):   # phase NQB, kf=1: tiles s+4, s+5 at slot s
        vt_sched[(NQB, s)] = [(1, s + 4), (1, s + 5)]

    # ---- Phase B: attention. Phase p: g = p // NQB, tb = p % NQB. ----
    ypairs = [None] * NPH
    scale = 1.0 / np.sqrt(D)

    def phase_gtb(p):
        return p // NQB, p % NQB

    def emit_qk(p, tk):
        g, tb = phase_gtb(p)
        qcol = g * T
        kcol = (2 + g) * T
        sc = []
        for i in range(2):
            p0 = i * 64
            sch = main_ps.tile([128, QB], F32, name="sc", tag="m")
            mm(
                sch[:],
                qk_sb[p0:p0 + 64, kcol + tk * 128: kcol + (tk + 1) * 128],
                qk_sb[p0:p0 + 64, qcol + tb * QB: qcol + (tb + 1) * QB],
                start=True,
                stop=True,
            )
            sc.append(sch)
        return sc

    # Schraudolph bf16 exp on the DVE for some slots per phase: bf16 bits of
    # exp(scale*s) ~= round(s*(scale*128/ln2) + (127*128 - 7.42)). The uniform
    # half-bit rounding bias cancels between numerator and denominator of the
    # softmax; residual noise is ~1.5% rms on the offloaded tiles.
    SCH_A = float(scale * 128.0 / np.log(2.0))
    SCH_B = float(127.0 * 128.0 - 7.42 + 0.5)
    DVE_SLOTS = (1, 4, 6, 9, 12, 15)

    def emit_exp(tk, sc):
        """exp of slot tk's scores, emitted one slot EARLY so the AV pair
        never head-of-line blocks the PE queue (keeps LDWEIGHTS hidden)."""
        halves = []
        if tk in DVE_SLOTS:
            et_i = exp_pool.tile([128, 2 * QB], I16, name="et", tag="et")
            for i in range(2):
                nc.vector.tensor_scalar(
                    et_i[:, i * QB:(i + 1) * QB], sc[i][:], SCH_A, SCH_B,
                    mybir.AluOpType.mult, mybir.AluOpType.add,
                )
                halves.append(et_i[:, i * QB:(i + 1) * QB].bitcast(BF16))
        else:
            et_b = exp_pool.tile([128, 2 * QB], BF16, name="et", tag="et")
            for i in range(2):
                nc.scalar.activation(
                    et_b[:, i * QB:(i + 1) * QB], sc[i][:],
                    mybir.ActivationFunctionType.Exp, scale=scale,
                )
                halves.append(et_b[:, i * QB:(i + 1) * QB])
        return halves

    def emit_av(p, tk, halves):
        g, tb = phase_gtb(p)
        for i in range(2):
            hh = 2 * g + i
            lhsT_v = v_sb[:, tk * HL * DE + hh * DE: tk * HL * DE + (hh + 1) * DE]
            mm(
                ypairs[p][i][0:DE, :],
                lhsT_v,
                halves[i],
                start=(tk == 0),
                stop=(tk == TT - 1),
            )

    norm_state = {}

    # Reciprocal off the DVE critical path: exponent-flip bit-trick seed on
    # the DVE (one cheap int op: 0x7EF311C7 - bits(x) == ((bits(x) XOR
    # 0x7FFFFFFF) - 0x010CEE38) for positive x), then one Newton-Raphson
    # round on the otherwise-idle GpSimd engine. Junk rows of dens2 stay 1.0
    # so no NaN/Inf can leak through the broadcast matmul. Seed max error
    # ~5%; after one NR round ~0.3% worst-case on the denominator.
    def emit_gp_recip(p):
        st = norm_state[p]
        seed = small_pool.tile([128, QB], I32, name="gpr")
        t0 = small_pool.tile([128, QB], F32, name="gpr")
        t1 = small_pool.tile([128, QB], F32R, name="gpr")
        nc.vector.tensor_scalar(
            seed[:], dens2[:].bitcast(I32), -1, 0x7EF311C7,
            mybir.AluOpType.mult, mybir.AluOpType.add,
        )
        r = seed[:].bitcast(F32)
        nc.gpsimd.tensor_tensor(t0[:], dens2[:].bitcast(F32), r, mybir.AluOpType.mult)
        nc.gpsimd.tensor_scalar(
            t0[:], t0[:], -1.0, 2.0, mybir.AluOpType.mult, mybir.AluOpType.add
        )
        with nc.allow_low_precision(reason="f32r moving operand for bc matmul"):
            nc.gpsimd.tensor_tensor(t1[:], r, t0[:], mybir.AluOpType.mult)
        st["rec_pre"] = t1[:]

    def emit_norm_step(p, step):
        """Normalization of phase p: park dens (DVE), GpSimd reciprocal,
        broadcast matmul of the reciprocals (PE), then yT multiplies (DVE)."""
        g, tb = phase_gtb(p)
        yp = ypairs[p]
        if step == 0:
            nc.vector.tensor_copy(dens2[0:1, :], yp[0][D:DE, :])
            nc.vector.tensor_copy(dens2[64:65, :], yp[1][D:DE, :])
            norm_state[p] = {}
        elif step == 1:
            emit_gp_recip(p)
        elif step == 2:
            bc = main_ps.tile([128, QB], F32, name="bc", tag="m")
            mm(bc[:], sel_sb[:], norm_state[p]["rec_pre"], start=True, stop=True)
            rec = small_pool.tile([128, QB], F32R, name="gpr")
            with nc.allow_low_precision(reason="f32r rounding for DVE mul"):
                nc.vector.tensor_copy(rec[:], bc[:])
            norm_state[p]["rec"] = rec
        else:
            rec = norm_state[p]["rec"]
            ycol = g * T + tb * QB
            for i in range(2):
                p0 = i * 64
                nc.vector.tensor_mul(
                    yT_sb[p0:p0 + 64, ycol: ycol + QB],
                    yp[i][0:D, :],
                    rec[p0:p0 + 64, :],
                )
            del norm_state[p]

    NORM_SLOTS = (0, 1, 7, 8)

    def emit_cproj_step(tb, j):
        """c_proj output tile tt = tb*4 + j: two 512-wide halves, then bf16
        staging copies (split ACT/DVE) and one DMA out."""
        tt = tb * (QB // 128) + j
        OSUB = 512
        osb = out_pool.tile([128, C], BF16, name="osb")
        # kk-outer: both 512-wide output halves consume each loaded yT
        # stationary back-to-back, so the second LDWEIGHTS is a cheap reload
        # of the same weights and the PE stream stays at ~216ns/MM.
        opss = [main_ps.tile([128, OSUB], F32, name="ops", tag="m") for _ in range(2)]
        for kk in range(KK):
            lhsT_y = yT_sb[:, kk * T + tt * 128: kk * T + (tt + 1) * 128]
            for ob in range(2):
                mm(
                    opss[ob][:],
                    lhsT_y,
                    wp_sb[:, kk * C + ob * OSUB: kk * C + (ob + 1) * OSUB],
                    start=(kk == 0),
                    stop=(kk == KK - 1),
                )
        nc.scalar.copy(osb[:, 0:OSUB], opss[0][:])
        nc.vector.tensor_copy(osb[:, OSUB:2 * OSUB], opss[1][:])
        nc.sync.dma_start(out[tt * 128:(tt + 1) * 128, :], osb[:])

    # cproj(tb) runs in phase NQB + tb + 1; cproj(NQB-1) lands in the tail.
    CPROJ_SLOTS = (9, 11, 13, 15)

    et_next = None
    sc_next = None
    for p in range(NPH):
        g, tb = phase_gtb(p)
        ypairs[p] = [y_ps.tile([128, QB], F32, name="yps", tag="y") for _ in range(2)]
        if p == 0:
            sc_next = emit_qk(0, 0)
            et_next = emit_exp(0, sc_next)
        for tk in range(TT):
            et_cur = et_next
            if tk < TT - 1:
                sc_next = emit_qk(p, tk + 1)
                et_next = emit_exp(tk + 1, sc_next)
            elif p < NPH - 1:
                sc_next = emit_qk(p + 1, 0)
                et_next = emit_exp(0, sc_next)
            else:
                et_next = None
            emit_av(p, tk, et_cur)
            for kf, tt in vt_sched.get((p, tk), ()):
                emit_vtrans(kf, tt)
            for kind, ft, u in unit_sched.get((p, tk), ()):
                emit_unit(kind, ft, u, copy_dve=(u % 2 == 0))
            if p >= 1 and tk in NORM_SLOTS:
                emit_norm_step(p - 1, NORM_SLOTS.index(tk))
            if p >= NQB + 1 and tk in CPROJ_SLOTS:
                emit_cproj_step(p - NQB - 1, CPROJ_SLOTS.index(tk))

    # ---- tail: normalize the last phase, final cproj tile set, with dummy
    # matmuls keeping the PE warm through the GpSimd/DVE-side chain. ----
    emit_norm_step(NPH - 1, 0)
    emit_norm_step(NPH - 1, 1)   # seed + GpSimd Newton round (~4us)
    new_dummy_tile()
    dummy_mm(20)
    emit_norm_step(NPH - 1, 2)   # bc matmul + rec copy
    dummy_mm(4)
    emit_norm_step(NPH - 1, 3)   # yT multiplies (DVE)
    dummy_mm(4)
    for j in range(4):
        emit_cproj_step(NQB - 1, j)
        if j < 3:
            dummy_mm(2)

    stack.close()


def build_nc(T=T_FULL, C=C_FULL):
    nc = bass.Bass("TRN2")
    CT = C // 128
    FTW = CT * 128
    xT = nc.dram_tensor("xT", [C, T], BF16, kind="ExternalInput")
    wqk = nc.dram_tensor("wqk", [128, 4 * FTW], BF16, kind="ExternalInput")
    wv = nc.dram_tensor("wv", [128, 2 * FTW], BF16, kind="ExternalInput")
    wp = nc.dram_tensor("wp", [128, (CLOC // 128) * C], BF16, kind="ExternalInput")
    sel = nc.dram_tensor("sel", [128, 128], F32R, kind="ExternalInput")
    zed = nc.dram_tensor("zed", [128, 512], F32R, kind="ExternalInput")
    ident = nc.dram_tensor("ident", [128, 128], BF16, kind="ExternalInput")
    out = nc.dram_tensor("out", [T, C], BF16, kind="ExternalOutput")
    with tile.TileContext(nc) as tc:
        emit_mha_kernel(tc, out[:], xT[:], wqk[:], wv[:], wp[:], sel[:], zed[:], ident[:], T, C)
    return legalize_waits(nc)


def _sbuf_tiled(w):
    """[K, F] -> [128, (K//128)*F] with per-128-row chunks laid side by side
    (the layout emit_mha_kernel indexes as [p, ct*F + f])."""
    K, F = w.shape
    CT = K // 128
    return np.ascontiguousarray(
        w.reshape(CT, 128, F).transpose(1, 0, 2).reshape(128, CT * F)
    )


def make_in_maps(x, W_attn, W_proj):
    """Host-side shard + layout prep for the 8 cores."""
    bf16 = mybir.dt.np(BF16)
    C = x.shape[2]
    sel = np.zeros((128, 128), np.float32)
    sel[0, 0:64] = 1.0
    sel[64, 64:128] = 1.0
    in_maps = []
    for core in range(N_CORES):
        b, hg = divmod(core, N_CORES // B)
        s0, s1 = hg * CLOC, (hg + 1) * CLOC
        Wq = W_attn[s0:s1, :]
        Wk = W_attn[C + s0:C + s1, :]
        Wv = W_attn[2 * C + s0:2 * C + s1, :]
        wqk_slices = [
            _sbuf_tiled(Wq[0:128, :].T), _sbuf_tiled(Wq[128:256, :].T),
            _sbuf_tiled(Wk[0:128, :].T), _sbuf_tiled(Wk[128:256, :].T),
        ]
        wv_slices = [
            _sbuf_tiled(Wv[0:128, :].T), _sbuf_tiled(Wv[128:256, :].T),
        ]
        in_maps.append({
            "sel": sel,
            "zed": np.ones((128, 512), np.float32),
            "ident": np.eye(128).astype(bf16),
            "xT": np.ascontiguousarray(x[b].T).astype(bf16),
            "wqk": np.concatenate(wqk_slices, axis=1).astype(bf16),
            "wv": np.concatenate(wv_slices, axis=1).astype(bf16),
            "wp": _sbuf_tiled(W_proj[:, s0:s1].T).astype(bf16),
        })
    return in_maps


_CACHED_NC = None


def kernel(x, W_attn, W_proj, b_proj, _trace=False):
    global _CACHED_NC
    x = np.asarray(x, dtype=np.float32)
    W_attn = np.asarray(W_attn, dtype=np.float32)
    W_proj = np.asarray(W_proj, dtype=np.float32)
    b_proj = np.asarray(b_proj, dtype=np.float32)

    if _CACHED_NC is None:
        _CACHED_NC = build_nc(T=x.shape[1], C=x.shape[2])
    nc = _CACHED_NC

    in_maps = make_in_maps(x, W_attn, W_proj)
    res = bass_utils.run_bass_kernel_spmd(
        nc, in_maps, core_ids=list(range(N_CORES)), trace=_trace,
    )
    parts = [np.asarray(r["out"], dtype=np.float32) for r in res.results]
    G = N_CORES // B
    out = np.stack(
        [np.sum(parts[b * G:(b + 1) * G], axis=0) + b_proj for b in range(B)], axis=0
    ).astype(np.float32)
    if _trace:
        return out, res
    return out


if __name__ == "__main__":
    nc = build_nc()
    print("built OK")
